# revision 51
# baseline (speedup 1.0000x reference)
"""Trainium2 Bass kernel for nn_BuddingLayer (moe_routing).

Computation (B=512, SIN=SOUT=2048, K=128 buds):
  dense = (x * ~mask) @ weight.T + bias          mask = one-hot(sat_idx)
  per bud k (v = x[:, sat_idx[k]]):
    h1 = relu(v * c1[k] + b1[k])                 c1[k,j] = sum_i W1[k,i,j]/3
    h2 = relu(h1 @ W2[k] + b2[k])                [B, 3]
    u += relu(h2 @ W3[k] + b3[k])                [B, 2048]
  out = dense + u

Sharding: output-feature split, 256 columns per core (8 cores), compute in
transposed layout [o_part, b_free].  Host does layout prep only (slices,
transposes, fp16 casts, w3/b3 packing); all x-dependent math runs on device.

Bud path: one bud per 32-row PE group; supertile t = buds {t, 32+t, 64+t,
96+t} (stride-32 grouping so the h2 pack is one flat-order DMA per j-row).
K=4 matmul per (bud, oc128) with a constant-1.0 4th rhs row whose lhsT row
carries b3 (bias folded into the MM).  One PSUM tile = one bud pair =
[128, 1024] (2 banks), ring of 4 fills PSUM.  Drain through three engines
in parallel (build-time greedy balances emitted busy-ns per engine):
  - ACT relu exits -> fp16 R tiles,
  - DVE scalar_tensor_tensor chains: S = relu(z) + S (fused relu+add),
  - Pool SWDGE accum-DMA chains absorb R tiles (desc-gen on the Pool
    engine, the adds on otherwise-idle DMA engines; GPSIMD has no
    TensorScalarPtr and no PSUM port on TRN2).
Dense accumulates in one long-lived PSUM ring slot (one chunk matmul pair
interleaved every couple of bud tiles), drained once to SBUF fp32.  PE
p-state warm-up matmuls abut the bud stream (idle gaps reset the clock
ramp).  oc half 0 retires a few supertiles early so its fold/store tail
overlaps oc half 1's exits.  Final out = (dense + bias) + buds is one DVE
STT per oc half (bias via the STT scalar operand), one store DMA per half.
"""

import numpy as np

N_CORES = 8
B = 512
SIN = 2048
SOUT = 2048
K = 128
OC = SOUT // N_CORES          # 256 output cols per core
NCHUNK = SIN // 128           # 16 contraction chunks for dense
NT = K // 4                   # 32 supertiles (4 buds each, stride-32 groups)

# cost-model rates (ns) for [128, 1024] tiles, used by the greedy balancer
_C_ACT_EXIT = 1038            # ACT relu exit, PSUM->fp16
_C_DVE_STT = 1192             # DVE STT relu+add, PSUM in0
_C_DVE_ADD = 594              # DVE fp16 tensor_tensor add
_C_POOL_ACC = 1038            # Pool SWDGE accum-DMA desc-gen

_CFG = {
    "nch": 4,              # pacc chains per oc
    "r_bufs": 6,           # R ring depth
    "backlog_act": 10,     # force-DVE exit when pend total >= this
    "act_stop": 0,         # no ACT exits when n_left <= this
    "act_end": 0,          # force-ACT exits when n_left <= this
    "td_tail_seed": 0,     # bias: DVE tail folds pre-charged
    "pool_eager": 4,       # force pool merge when pend >= this
    "dve_merge": 6,        # DVE TT merge when pend >= this
    "retire_at": 0,        # fold upper chains when n_left == this (0=never)
    "retire_to": 2,        # chains remaining after retirement
    "pool_stop": 0,        # no pool merges when target oc n_left < this
    "warm_n": 8,
    "stag": 4,
    "dense_dma_tiles": (4, 18),
    "dense_spread": (24, 2),
}

_compiled = {}


def _build(chunk_status):
    import concourse.bacc as bacc
    import concourse.mybir as mybir
    import concourse.tile as tile

    f32, f16 = mybir.dt.float32, mybir.dt.float16
    AL = mybir.AluOpType
    AF = mybir.ActivationFunctionType

    live = [c for c in range(NCHUNK) if chunk_status[c] != "full"]
    NL = len(live)

    nc = bacc.Bacc("TRN2", target_bir_lowering=False, debug=False,
                   num_devices=N_CORES)

    # ---- DRAM I/O (per core) ----
    hdr = nc.dram_tensor("hdr", [K, 24], f32, kind="ExternalInput")
    xsatd = nc.dram_tensor("xsatd", [K, B], f16, kind="ExternalInput")
    w3bT = nc.dram_tensor("w3bT", [128, OC * NT], f16, kind="ExternalInput")
    onesd = nc.dram_tensor("onesd", [4, 512 * NT], f16, kind="ExternalInput")
    x16d = nc.dram_tensor("x16d", [128, 512 * NL], f16, kind="ExternalInput")
    w16d = nc.dram_tensor("w16d", [128, OC * NL], f16, kind="ExternalInput")
    biascd = nc.dram_tensor("biascd", [128, 2], f32, kind="ExternalInput")
    outT = nc.dram_tensor("outT", [OC, B], f32, kind="ExternalOutput")

    with tile.TileContext(nc) as tc:
        with (
            tc.tile_pool(name="const", bufs=1) as cp,
            tc.tile_pool(name="rtiles", bufs=6) as rp,
            tc.tile_pool(name="chains", bufs=2) as chp,
            tc.tile_pool(name="psum", bufs=4, space="PSUM") as pp,
        ):
            # ---------- early constants ----------
            # ACT relu table warm
            wa = cp.tile([1, 1], f32)
            nc.vector.memset(wa[:], 1.0)
            warm_act = cp.tile([1, 1], f32)
            nc.scalar.activation(warm_act[:], wa[:], AF.Relu)

            hdrs = cp.tile([K, 24], f32)
            nc.sync.dma_start(hdrs[:], hdr.ap())
            xsat = cp.tile([K, B], f16)
            nc.sync.dma_start(xsat[:], xsatd.ap())
            h2t = cp.tile([128, 512 * NT], f16)
            nc.sync.dma_start(h2t[3::32, :], onesd.ap())
            w3b = cp.tile([128, OC * NT], f16)
            q = OC * NT // 4
            nc.sync.dma_start(w3b[:, 0:q], w3bT.ap()[:, 0:q])

            # PE p-state warm: dummy matmuls gated on the onesd DMA so they
            # abut the bud stream (idle gaps reset the tensor engine's clock
            # ramp).  lhsT must start at partition 0 -> stage one ones row.
            wrow = cp.tile([1, 512], f16)
            nc.sync.dma_start(wrow[:], onesd.ap()[0:1, 0:512])
            warm_ps = pp.tile([128, 1024], f32, tag="zps", name="warmps")
            for i in range(_CFG["warm_n"]):
                nc.tensor.matmul(warm_ps[:, 0:512], wrow[0:1, 0:128],
                                 wrow[0:1, 0:512], start=True, stop=True,
                                 tile_position=(0, 0))

            # c1[k, j] = (W1[k,0,j] + W1[k,1,j] + W1[k,2,j]) / 3
            w1s = hdrs[:, 0:9]
            c1a = cp.tile([K, 3], f32)
            nc.vector.tensor_tensor(c1a[:], w1s[:, 0:3], w1s[:, 3:6], AL.add)
            c1b = cp.tile([K, 3], f32)
            nc.vector.tensor_tensor(c1b[:], c1a[:], w1s[:, 6:9], AL.add)
            c1s = cp.tile([K, 3], f32)
            nc.vector.tensor_scalar_mul(c1s[:], c1b[:], 1.0 / 3.0)

            # ---------- h path: v -> h1 -> h2 (layout [k, b]) ----------
            h1 = cp.tile([K, 3 * B], f16)
            for j in (0, 2):
                nc.scalar.activation(h1[:, B * j:B * (j + 1)], xsat[:],
                                     AF.Relu, bias=hdrs[:, 9 + j:10 + j],
                                     scale=c1s[:, j:j + 1])
            h1p = rp.tile([K, B], f16, tag="hscr", bufs=3, name="h1p")
            nc.vector.tensor_scalar(h1p[:], xsat[:], c1s[:, 1:2],
                                    hdrs[:, 10:11], AL.mult, AL.add)
            nc.vector.tensor_scalar_max(h1[:, B:2 * B], h1p[:], 0.0)
            # h2_pre[j] = sum_i h1[i] * W2[:, i, j];  W2 col (3i+j) at 12+3i+j
            h2 = cp.tile([K, 3 * B], f16)
            for j in range(3):
                m0 = rp.tile([K, B], f16, tag="hscr", bufs=3, name=f"h2m{j}")
                nc.vector.tensor_scalar_mul(m0[:], h1[:, 0:B],
                                            hdrs[:, 12 + j:13 + j])
                m1 = rp.tile([K, B], f16, tag="hsc2", bufs=3, name=f"h2n{j}")
                nc.vector.tensor_scalar_mul(m1[:], h1[:, B:2 * B],
                                            hdrs[:, 15 + j:16 + j])
                m2 = rp.tile([K, B], f16, tag="hsc3", bufs=3, name=f"h2o{j}")
                nc.scalar.activation(m2[:], h1[:, 2 * B:3 * B], AF.Copy,
                                     scale=hdrs[:, 18 + j:19 + j])
                a0 = rp.tile([K, B], f16, tag="hscr", bufs=3, name=f"h2a{j}")
                nc.vector.tensor_tensor(a0[:], m0[:], m1[:], AL.add)
                a1 = rp.tile([K, B], f16, tag="hsc2", bufs=3, name=f"h2b{j}")
                nc.vector.tensor_tensor(a1[:], a0[:], m2[:], AL.add)
                nc.scalar.activation(h2[:, B * j:B * (j + 1)], a1[:], AF.Relu,
                                     bias=hdrs[:, 21 + j:22 + j])

            # ---------- pack h2 -> h2t (one DMA per j, flat-order pairing):
            # h2t[32g+j, 512t+b] = h2[32g+t, 512j+b] ----------
            for j in range(3):
                nc.sync.dma_start(
                    h2t[j::32, :].rearrange("p (t b) -> p t b", b=512),
                    h2[:, B * j:B * (j + 1)])
            # rest of w3b arrives while the first supertiles run
            for qi in range(1, 4):
                nc.sync.dma_start(w3b[:, q * qi:q * (qi + 1)],
                                  w3bT.ap()[:, q * qi:q * (qi + 1)])

            # dense inputs: DMAs deferred into the bud loop
            x16 = cp.tile([128, 512 * NL], f16)
            w16 = cp.tile([128, OC * NL], f16)
            biasc = cp.tile([128, 2], f32)
            dense_dmas = [0]

            def emit_dense_dmas():
                # one or two stages depending on _CFG["dense_dma_tiles"]
                st = dense_dmas[0]
                stages = len(_CFG["dense_dma_tiles"])
                if st >= stages:
                    return
                dense_dmas[0] = st + 1
                h = NL // 2 if stages == 2 else NL
                if st == 0:
                    nc.sync.dma_start(x16[:, 0:512 * h], x16d.ap()[:, 0:512 * h])
                    nc.sync.dma_start(w16[:, 0:OC * h], w16d.ap()[:, 0:OC * h])
                    nc.sync.dma_start(biasc[:], biascd.ap())
                else:
                    nc.sync.dma_start(x16[:, 512 * h:], x16d.ap()[:, 512 * h:])
                    nc.sync.dma_start(w16[:, OC * h:], w16d.ap()[:, OC * h:])

            # ---------- bud loop state ----------
            tA = [5000.0]
            tD = [5500.0 + _CFG["td_tail_seed"]]
            tP = [8000.0]
            pend = {0: [], 1: []}
            r_open = {0: None, 1: None}
            dchain = {0: None, 1: None}
            nR = [0]
            nS = {0: 0, 1: 0}
            nPm = [0]
            nM = [0]

            NCH = _CFG["nch"]
            pacc = {(oc, i): cp.tile([128, 1024], f16, name=f"pacc{oc}_{i}")
                    for oc in range(2) for i in range(NCH)}
            pacc_started = {k: False for k in pacc}
            pacc_rr = {0: 0, 1: 0}
            nch_act = {0: NCH, 1: NCH}

            def emit_pool_merge(oc):
                """Absorb one pending tile into a pacc chain via SWDGE
                accum-DMA (desc-gen on Pool engine, add on DMA engines)."""
                if not pend[oc]:
                    return False
                r = pend[oc].pop(0)
                i = pacc_rr[oc]
                pacc_rr[oc] = (i + 1) % nch_act[oc]
                if not pacc_started[(oc, i)]:
                    nc.gpsimd.dma_start(pacc[(oc, i)][:], r)      # init copy
                    pacc_started[(oc, i)] = True
                else:
                    nc.gpsimd.dma_start(pacc[(oc, i)][:], r, accum_op=AL.add)
                tP[0] += _C_POOL_ACC
                nPm[0] += 1
                return True

            def emit_dve_merge(oc):
                if len(pend[oc]) < 2:
                    return False
                r1 = pend[oc].pop(0)
                r2 = pend[oc].pop(0)
                m = rp.tile([128, 1024], f16, tag=f"M{oc}", bufs=2,
                            name=f"M{oc}_{nM[0]}")
                nc.vector.tensor_tensor(m[:], r1, r2, AL.add)
                # front of queue: M ring slots must be consumed before wrap
                pend[oc].insert(0, m[:])
                tD[0] += _C_DVE_ADD
                nM[0] += 1
                return True

            # ---------- dense: spread chunks into one long-lived ring tile
            # (one chunk per couple of bud tiles -> no PE stall bursts) ----
            dsb = cp.tile([128, 1024], f32)       # SBUF dense accumulator
            dense_state = {"tile": None, "ci": 0}

            def emit_dense_chunk():
                ci = dense_state["ci"]
                if ci >= NL:
                    return
                if dense_state["tile"] is None:
                    dense_state["tile"] = pp.tile([128, 1024], f32, tag="zps",
                                                  name="densacc")
                bt = dense_state["tile"]
                for h in range(2):
                    nc.tensor.matmul(
                        bt[:, 512 * h:512 * (h + 1)],
                        w16[:, OC * ci + 128 * h: OC * ci + 128 * h + 128],
                        x16[:, 512 * ci:512 * (ci + 1)],
                        start=(ci == 0), stop=(ci == NL - 1))
                dense_state["ci"] = ci + 1
                if dense_state["ci"] == NL:
                    nc.vector.tensor_copy(dsb[:], bt[:, 0:1024])
                    tD[0] += 1192

            # ---------- tile order: oc0 retires early ----------
            STAG = _CFG["stag"]
            order = [(t, oc, u) for t in range(NT - STAG)
                     for oc in range(2) for u in range(2)]
            order += [(t, oc, u) for oc in range(2)
                      for t in range(NT - STAG, NT) for u in range(2)]
            n_left = {0: 2 * NT, 1: 2 * NT}

            outsb = cp.tile([128, 1024], f32)

            def emit_tail(oc):
                """Fold chains + pends, add dense + bias, store this half."""
                parts = list(pend[oc])
                pend[oc] = []
                if dchain[oc] is not None:
                    parts.append(dchain[oc])
                    dchain[oc] = None
                for i in range(NCH):
                    if pacc_started[(oc, i)]:
                        parts.append(pacc[(oc, i)][:])
                # pairwise tree fold; ring sized to the part count so the
                # ring can never wrap onto an unconsumed slot
                ti = 0
                fbufs = max(2, len(parts))
                while len(parts) > 1:
                    a = parts.pop(0)
                    b = parts.pop(0)
                    m = rp.tile([128, 1024], f16, tag=f"F{oc}", bufs=fbufs,
                                name=f"F{oc}_{ti}")
                    nc.vector.tensor_tensor(m[:], a, b, AL.add)
                    parts.append(m[:])
                    ti += 1
                    tD[0] += _C_DVE_ADD
                tot = rp.tile([128, 512], f16, tag=f"T{oc}", bufs=1,
                              name=f"T{oc}")
                nc.vector.tensor_tensor(tot[:], parts[0][:, 0:512],
                                        parts[0][:, 512:1024], AL.add)
                for bh in range(2):
                    sl = slice(512 * oc + 256 * bh, 512 * oc + 256 * (bh + 1))
                    nc.vector.scalar_tensor_tensor(
                        outsb[:, sl], dsb[:, sl], biasc[:, oc:oc + 1],
                        tot[:, 256 * bh:256 * (bh + 1)], AL.add, AL.add)
                    nc.sync.dma_start(
                        outT.ap()[128 * oc:128 * (oc + 1),
                                  256 * bh:256 * (bh + 1)],
                        outsb[:, sl])

            # ---------- main loop ----------
            tile_idx = 0
            for (t, oc, u) in order:
                zps = pp.tile([128, 1024], f32, tag="zps",
                              name=f"z{t}_{oc}_{u}")
                for d in range(2):
                    g = 2 * u + d
                    nc.tensor.matmul(
                        zps[:, 512 * d:512 * (d + 1)],
                        w3b[32 * g:32 * g + 4,
                            OC * t + 128 * oc: OC * t + 128 * oc + 128],
                        h2t[32 * g:32 * g + 4, 512 * t:512 * (t + 1)],
                        start=True, stop=True, tile_position=(32 * g, 0))
                if tile_idx in _CFG["dense_dma_tiles"]:
                    emit_dense_dmas()
                ds, de = _CFG["dense_spread"]
                if tile_idx >= ds and (tile_idx - ds) % de == 0:
                    emit_dense_chunk()
                # exit: ACT or DVE STT chain, greedily balanced; near this
                # half's retirement or with a merge backlog prefer DVE
                backlog = len(pend[0]) + len(pend[1])
                if tile_idx < 4:
                    use_act = (tile_idx % 2 == 1)
                elif n_left[oc] <= _CFG["act_end"]:
                    # endgame: ACT exit + a 594ns tail fold beats loading the
                    # critical-path DVE with a 1192ns STT
                    use_act = backlog < _CFG["backlog_act"] + 4
                else:
                    use_act = (tA[0] + _C_ACT_EXIT <= tD[0] + _C_DVE_STT
                               and backlog < _CFG["backlog_act"]
                               and n_left[oc] > _CFG["act_stop"])
                if use_act:
                    r = rp.tile([128, 1024], f16, tag=f"R{oc}",
                                bufs=_CFG["r_bufs"], name=f"R{oc}_{nR[0]}")
                    nc.scalar.activation(r[:], zps[:], AF.Relu)
                    pend[oc].append(r[:])
                    tA[0] += _C_ACT_EXIT
                    nR[0] += 1
                else:
                    s = chp.tile([128, 1024], f16, tag=f"S{oc}", bufs=2,
                                 name=f"S{oc}_{nS[oc]}")
                    if dchain[oc] is None:
                        nc.vector.tensor_scalar_max(s[:], zps[:], 0.0)
                    else:
                        nc.vector.scalar_tensor_tensor(
                            s[:], zps[:], 0.0, dchain[oc], AL.max, AL.add)
                    dchain[oc] = s[:]
                    tD[0] += _C_DVE_STT
                    nS[oc] += 1
                # merges
                while True:
                    mx = max(tA[0], tD[0], tP[0])
                    moc = oc if pend[oc] else 1 - oc
                    if n_left[moc] >= _CFG["pool_stop"] \
                            and (tP[0] + _C_POOL_ACC <= mx
                                 or len(pend[moc]) >= _CFG["pool_eager"]) \
                            and emit_pool_merge(moc):
                        continue
                    if tD[0] + _C_DVE_ADD <= mx and len(pend[moc]) >= _CFG["dve_merge"] \
                            and emit_dve_merge(moc):
                        continue
                    break
                n_left[oc] -= 1
                if n_left[oc] == _CFG["retire_at"] and nch_act[oc] == NCH:
                    # retire upper chains while the last exits run
                    keep = _CFG["retire_to"]
                    nch_act[oc] = keep
                    pacc_rr[oc] = 0
                    for ic in range(keep, NCH):
                        if pacc_started[(oc, ic)]:
                            nc.gpsimd.dma_start(pacc[(oc, ic % keep)][:],
                                                pacc[(oc, ic)][:],
                                                accum_op=AL.add)
                            pacc_started[(oc, ic)] = False
                            tP[0] += _C_POOL_ACC
                if n_left[oc] == 0:
                    while dense_state["ci"] < NL:
                        emit_dense_chunk()
                    emit_tail(oc)
                tile_idx += 1
    nc.finalize()
    return nc


def _prep_inputs(x, sat_idx, weight, bias, W1, b1, W2, b2, W3, b3):
    """Host-side shard/layout prep. Returns (chunk_status, per-core inputs)."""
    x = np.asarray(x, np.float32)
    sat = np.asarray(sat_idx).astype(np.int64)
    weight = np.asarray(weight, np.float32)
    bias = np.asarray(bias, np.float32)
    W1 = np.asarray(W1, np.float32).reshape(K, 9)
    b1 = np.asarray(b1, np.float32)
    W2 = np.asarray(W2, np.float32).reshape(K, 9)
    b2 = np.asarray(b2, np.float32)
    W3 = np.asarray(W3, np.float32)
    b3 = np.asarray(b3, np.float32)

    mask = np.ones(SIN, np.float32)
    mask[sat] = 0.0
    chunk_status = []
    for c in range(NCHUNK):
        mc = mask[128 * c:128 * (c + 1)]
        if not mc.any():
            chunk_status.append("full")
        elif mc.all():
            chunk_status.append("clean")
        else:
            chunk_status.append("partial")
    chunk_status = tuple(chunk_status)
    live = [c for c in range(NCHUNK) if chunk_status[c] != "full"]

    # hdr: w1 | b1 | w2 | b2
    hdr = np.empty((K, 24), np.float32)
    hdr[:, 0:9] = W1
    hdr[:, 9:12] = b1
    hdr[:, 12:21] = W2
    hdr[:, 21:24] = b2
    hdr = np.ascontiguousarray(hdr)
    xsat16 = np.ascontiguousarray(x[:, sat].T.astype(np.float16))

    onesd = np.ones((4, 512 * NT), np.float16)

    # dense inputs: masked columns zeroed on host (only for partial chunks),
    # fully-masked chunks dropped
    xm = x if all(s != "partial" for s in chunk_status) else x * mask[None, :]
    xT16 = xm.T.astype(np.float16)                  # [SIN, B]
    x16d = np.ascontiguousarray(
        np.stack([xT16[128 * c:128 * (c + 1), :] for c in live], axis=1)
        .reshape(128, len(live) * B))

    in_maps = []
    for c in range(N_CORES):
        sl = slice(OC * c, OC * (c + 1))
        # w3b: row 32g+j, col 256t+o:  j<3 -> W3[32g+t, j, sl][o]; j=3 -> b3
        w3c = W3[:, :, sl]                          # [K, 3, OC]
        b3c = b3[:, sl]                             # [K, OC]
        w3b = np.empty((128, OC * NT), np.float32)
        kk = np.arange(NT)
        for g in range(4):
            rows = w3c[32 * g + kk]                 # [NT, 3, OC]
            for j in range(3):
                w3b[32 * g + j] = rows[:, j, :].reshape(-1)
            w3b[32 * g + 3] = b3c[32 * g + kk].reshape(-1)
        wT16 = weight[sl, :].T.astype(np.float16)   # [SIN, OC]
        w16d = np.ascontiguousarray(
            np.stack([wT16[128 * cc:128 * (cc + 1), :] for cc in live], axis=1)
            .reshape(128, len(live) * OC))
        biasc = np.ascontiguousarray(bias[sl].reshape(2, 128).T)  # [128, 2]
        in_maps.append({
            "hdr": hdr,
            "xsatd": xsat16,
            "w3bT": np.ascontiguousarray(w3b.astype(np.float16)),
            "onesd": onesd,
            "x16d": x16d,
            "w16d": w16d,
            "biascd": biasc,
        })
    return chunk_status, in_maps


def kernel(**inputs) -> np.ndarray:
    from concourse.bass_utils import run_bass_kernel_spmd

    chunk_status, in_maps = _prep_inputs(
        inputs["x"], inputs["sat_idx"], inputs["weight"], inputs["bias"],
        inputs["W1"], inputs["b1"], inputs["W2"], inputs["b2"],
        inputs["W3"], inputs["b3"],
    )
    if chunk_status not in _compiled:
        _compiled[chunk_status] = _build(chunk_status)
    nc = _compiled[chunk_status]
    res = run_bass_kernel_spmd(nc, in_maps, core_ids=list(range(N_CORES)))
    outT = np.concatenate([res.results[c]["outT"] for c in range(N_CORES)], axis=0)
    return np.ascontiguousarray(outT.T).astype(np.float32)
